# revision 13
# baseline (speedup 1.0000x reference)
"""Capsule dynamic-routing kernel for 8 TRN2 NeuronCores.

Problem: u_ji = einsum('bnd,node->bnoe', u_i, W[0]) + bias ; r=3 rounds of
dynamic routing (softmax over n_out, weighted sum over n_in, squash, agreement
update) ; returns v (batch, n_out, d_out) float32.

Sharding: data-parallel over batch (32 per core), weights replicated.

Device design notes (baseline):
  * n is decomposed as n = 48*t + 12*j + 3*i + nl  (t<24, j<4, i<4, nl<3).
  * The projection runs on TensorE in 32x32 tiling mode. Stationary operand is
    u_i packed per (t,j,i)-group of 3 n's with an augmented contraction row
    d=8 equal to 1.0 so the bias is folded into the matmul. The moving operand
    is a host-materialized block-diagonal W slice. PSUM partitions are (j, b);
    evac writes u_ji to SBUF as bf16 "Lb" [128=(j,b), t, i, nl*160].
  * Iteration 0 uses uniform coupling coefficients, so s_0 is computed exactly
    with one flat K=10368 matmul chain over (n, d9) without touching u_ji.
  * Routing (softmax over o, weighted sums over n, agreement over e) runs on
    DVE/ACT in the (j,b)-partition layout; the 4-way partition fold of
    s-partials and the 4-way replication of v use tiny exact fp32 matmuls with
    0/1 matrices.
"""

import os
import sys
import numpy as np

for _p in ("/opt/trn_rl_repo", "/root/.axon_site/_ro/trn_rl_repo"):
    if os.path.isdir(_p) and _p not in sys.path:
        sys.path.append(_p)

import ml_dtypes

BF16 = ml_dtypes.bfloat16

BATCH, N_IN, D_IN, N_OUT, D_OUT = 256, 1152, 8, 10, 16
R_ITERS = 3
N_CORES = 8
B = BATCH // N_CORES  # 32
D9 = D_IN + 1  # augmented contraction row carries the bias
NT, NJ, NI, NL = 24, 4, 4, 3  # n = 48t + 12j + 3i + nl
OE = N_OUT * D_OUT  # 160
KC = (N_IN * D9) // 128  # 81 flat contraction chunks for s_0
TCH = 2  # t-chunk size for the routing sweeps
NCH = NT // TCH

LAST_EXEC_NS = None

_BUILT = None


def _ensure_ntff_hook():
    """Dev-only: register the NTFF profile hook that bass_utils expects when
    trace=True under axon, if the agent image's antenv lacks it."""
    try:
        from antenv.axon_hooks import get_axon_ntff_profile_hook  # noqa: F401

        return
    except ImportError:
        pass
    import contextlib
    import ctypes
    import types

    so_path = "/opt/axon/libaxon_pjrt.so"
    if not os.path.exists(so_path):
        return
    try:
        lib = ctypes.CDLL(so_path)
        if not hasattr(lib, "axon_start_nrt_profile"):
            return
    except OSError:
        return
    lib.axon_start_nrt_profile.argtypes = [
        ctypes.POINTER(ctypes.c_int64),
        ctypes.c_size_t,
    ]
    lib.axon_start_nrt_profile.restype = ctypes.c_int64
    lib.axon_stop_nrt_profile.argtypes = [ctypes.c_char_p]
    lib.axon_stop_nrt_profile.restype = ctypes.c_int64

    @contextlib.contextmanager
    def _hook(output_dir, device_ids):
        import jax

        jax.devices()
        if device_ids:
            ids = (ctypes.c_int64 * len(device_ids))(*device_ids)
            rc = lib.axon_start_nrt_profile(ids, len(device_ids))
        else:
            rc = lib.axon_start_nrt_profile(None, 0)
        if rc != 0:
            raise RuntimeError(f"axon_start_nrt_profile rc={rc}")
        try:
            yield
        finally:
            n = lib.axon_stop_nrt_profile(str(output_dir).encode())
            print(f"ntff profile: {n} file(s) written to {output_dir}", file=sys.stderr)

    mod = types.ModuleType("antenv.axon_hooks")
    mod._HOOK = _hook
    mod.set_axon_ntff_profile_hook = lambda h: setattr(mod, "_HOOK", h)
    mod.get_axon_ntff_profile_hook = lambda: mod._HOOK
    sys.modules["antenv.axon_hooks"] = mod
    try:
        import antenv

        antenv.axon_hooks = mod
    except ImportError:
        pass


def _host_pack(u_i, weight, bias):
    """Builds the replicated operands (everything except per-core u_i packs)."""
    W = np.asarray(weight)[0]  # (n, o, d, e)
    bias2 = np.asarray(bias)[:, :, 0]  # (n, o)

    # w_bd[32i + 9nl + d, t, j, nl2*160 + o*16 + e]
    Wr = W.reshape(NT, NJ, NI, NL, N_OUT, D_IN, D_OUT)
    br = bias2.reshape(NT, NJ, NI, NL, N_OUT)
    w_bd = np.zeros((NI, 32, NT, NJ, NL, OE), np.float32)
    for nl in range(NL):
        # rows 9*nl + d ; only the nl2 == nl column block is non-zero
        blk = Wr[:, :, :, nl].transpose(2, 4, 0, 1, 3, 5)  # (i, d, t, j, o, e)
        w_bd[:, 9 * nl: 9 * nl + 8, :, :, nl, :] = blk.reshape(NI, 8, NT, NJ, OE)
        bb = br[:, :, :, nl].transpose(2, 0, 1, 3)  # (i, t, j, o)
        w_bd[:, 9 * nl + 8, :, :, nl, :] = np.broadcast_to(
            bb[..., None], (NI, NT, NJ, N_OUT, D_OUT)
        ).reshape(NI, NT, NJ, OE)
    w_bd = w_bd.reshape(128, NT, NJ * NL * OE).astype(BF16)

    # w_flat[k % 128, k // 128, o*16 + e] with k = 9n + d
    wf = np.concatenate(
        [
            W.transpose(0, 2, 1, 3).reshape(N_IN, D_IN, OE),
            np.broadcast_to(bias2[:, None, :, None], (N_IN, 1, N_OUT, D_OUT)).reshape(
                N_IN, 1, OE
            ),
        ],
        axis=1,
    ).reshape(N_IN * D9, OE)
    w_flat = wf.reshape(KC, 128, OE).transpose(1, 0, 2).astype(BF16)

    e_fold = np.zeros((128, B), np.float32)
    for j in range(NJ):
        e_fold[32 * j + np.arange(B), np.arange(B)] = 1.0
    e_rep = np.zeros((B, 128), np.float32)
    for r in range(NJ):
        e_rep[np.arange(B), 32 * r + np.arange(B)] = 1.0

    return w_bd, w_flat, e_fold, e_rep


def _host_pack_core(u_core):
    """Per-core u_i packs. u_core: (B, n, d) float32."""
    uir = u_core.reshape(B, NT, NJ, NI, NL, D_IN)
    u_pack = np.zeros((NI, 32, NT, NJ, B), np.float32)
    blk = uir.transpose(3, 4, 5, 1, 2, 0)  # (i, nl, d, t, j, b)
    for nl in range(NL):
        u_pack[:, 9 * nl: 9 * nl + 8] = blk[:, nl]
        u_pack[:, 9 * nl + 8] = 1.0
    u_pack = u_pack.reshape(128, NT, NJ, B).astype(BF16)

    uf = np.concatenate(
        [u_core.transpose(1, 2, 0), np.ones((N_IN, 1, B), np.float32)], axis=1
    ).reshape(N_IN * D9, B)
    u_flat = uf.reshape(KC, 128, B).transpose(1, 0, 2).astype(BF16)
    return u_pack, u_flat


def _build_bass():
    import concourse.mybir as mybir
    from concourse import bacc, tile

    dt = mybir.dt
    Alu = mybir.AluOpType
    Act = mybir.ActivationFunctionType
    Ax = mybir.AxisListType

    nc = bacc.Bacc(
        "TRN2", target_bir_lowering=False, debug=False, num_devices=N_CORES
    )

    p_upack = nc.declare_dram_parameter("u_pack", [128, NT, NJ, B], dt.bfloat16, isOutput=False)
    p_uflat = nc.declare_dram_parameter("u_flat", [128, KC, B], dt.bfloat16, isOutput=False)
    p_wbd = nc.declare_dram_parameter("w_bd", [128, NT, NJ * NL * OE], dt.bfloat16, isOutput=False)
    p_wflat = nc.declare_dram_parameter("w_flat", [128, KC, OE], dt.bfloat16, isOutput=False)
    p_efold = nc.declare_dram_parameter("e_fold", [128, B], dt.float32, isOutput=False)
    p_erep = nc.declare_dram_parameter("e_rep", [B, 128], dt.float32, isOutput=False)
    p_out = nc.declare_dram_parameter("v_out", [B, OE], dt.float32, isOutput=True)

    with tile.TileContext(nc) as tc:
        with (
            tc.tile_pool(name="const", bufs=1) as cpool,
            tc.tile_pool(name="wbd", bufs=3) as wpool,
            tc.tile_pool(name="z", bufs=2) as zpool,
            tc.tile_pool(name="small", bufs=2) as spool,
            tc.tile_pool(name="pproj", bufs=2, space="PSUM") as pp,
            tc.tile_pool(name="psmall", bufs=2, space="PSUM") as psml,
        ):
            # ---- resident inputs ----
            u_pack = cpool.tile([128, NT, NJ, B], dt.bfloat16)
            u_flat = cpool.tile([128, KC, B], dt.bfloat16)
            w_flat = cpool.tile([128, KC, OE], dt.bfloat16)
            e_fold = cpool.tile([128, B], dt.float32)
            e_rep = cpool.tile([B, 128], dt.float32)
            nc.sync.dma_start(out=u_pack[:], in_=p_upack[:])
            nc.sync.dma_start(out=u_flat[:], in_=p_uflat[:])
            nc.sync.dma_start(out=w_flat[:], in_=p_wflat[:])
            nc.sync.dma_start(out=e_fold[:], in_=p_efold[:])
            nc.sync.dma_start(out=e_rep[:], in_=p_erep[:])

            # ---- state ----
            Lb = cpool.tile([128, NT, NI, NL * OE], dt.bfloat16)
            NPP = NT * NI * NL  # 288 n-values per partition
            CH = TCH * NI * NL  # 24 n-values per routing chunk
            bb = cpool.tile([128, NPP, N_OUT], dt.float32)
            expb = cpool.tile([128, NPP, N_OUT], dt.bfloat16)
            cc = cpool.tile([128, NPP, N_OUT], dt.bfloat16)
            zsum = cpool.tile([128, NPP], dt.float32)
            zrec = cpool.tile([128, NPP], dt.float32)
            zrecb = cpool.tile([128, NPP], dt.bfloat16)
            s_part = cpool.tile([128, OE], dt.float32)
            s_sb = cpool.tile([B, OE], dt.float32)
            ssq = cpool.tile([B, OE], dt.float32)
            qn = cpool.tile([B, N_OUT], dt.float32)
            sqn = cpool.tile([B, N_OUT], dt.float32)
            den = cpool.tile([B, N_OUT], dt.float32)
            rden = cpool.tile([B, N_OUT], dt.float32)
            fac = cpool.tile([B, N_OUT], dt.float32)
            v_sb = cpool.tile([B, OE], dt.float32)
            vrep = cpool.tile([128, OE], dt.bfloat16)

            nc.vector.memset(bb[:], 0.0)

            # ---- s_0 = 0.1 * sum_n u_ji  (flat K = 10368 contraction) ----
            ps_s0 = psml.tile([128, 512], dt.float32, tag="s0")
            for k in range(KC):
                nc.tensor.matmul(
                    ps_s0[0:B, 0:OE],
                    u_flat[:, k, :],
                    w_flat[:, k, :],
                    start=(k == 0),
                    stop=(k == KC - 1),
                )

            # ---- projection: u_ji -> Lb ----
            for t in range(NT):
                wbd_t = wpool.tile([128, NJ * NL * OE], dt.bfloat16)
                nc.sync.dma_start(out=wbd_t[:], in_=p_wbd[:, t, :])
                wv = wbd_t[:].rearrange("p (j f) -> p j f", j=NJ)
                for ih in range(2):
                    ps = pp.tile([128, 2, 512], dt.float32)
                    for j in range(NJ):
                        for il in range(2):
                            i = 2 * ih + il
                            nc.tensor.matmul(
                                ps[32 * j: 32 * j + 32, il, 0: NL * OE],
                                u_pack[32 * i: 32 * i + 32, t, j, :],
                                wv[32 * i: 32 * i + 32, j, :],
                                start=True,
                                stop=True,
                                tile_position=(32 * i, 32 * j),
                            )
                    eng = nc.vector if (2 * t + ih) % 2 == 0 else nc.scalar
                    if eng is nc.vector:
                        nc.vector.tensor_copy(
                            Lb[:, t, 2 * ih: 2 * ih + 2, :], ps[:, :, 0: NL * OE]
                        )
                    else:
                        nc.scalar.copy(
                            Lb[:, t, 2 * ih: 2 * ih + 2, :], ps[:, :, 0: NL * OE]
                        )

            # ---- routing iterations ----
            for it in range(R_ITERS):
                if it == 0:
                    s_psum = ps_s0
                    s_scale = 0.1
                else:
                    # softmax over o (free dim), then s = sum_n c * u_ji
                    nc.scalar.activation(expb[:], bb[:], Act.Exp)
                    nc.vector.tensor_reduce(zsum[:], expb[:], axis=Ax.X, op=Alu.add)
                    nc.vector.reciprocal(zrec[:], zsum[:])
                    nc.vector.tensor_copy(zrecb[:], zrec[:])
                    nc.vector.tensor_mul(
                        cc[:],
                        expb[:],
                        zrecb[:, :, None].broadcast_to([128, NPP, N_OUT]),
                    )
                    nc.vector.memset(s_part[:], 0.0)
                    for ch in range(NCH):
                        ts = slice(TCH * ch, TCH * ch + TCH)
                        cs = slice(CH * ch, CH * ch + CH)
                        z = zpool.tile([128, CH, N_OUT, D_OUT], dt.bfloat16)
                        nc.vector.tensor_mul(
                            z[:],
                            Lb[:, ts].rearrange(
                                "p t i (nl o e) -> p (t i nl) o e", nl=NL, o=N_OUT
                            ),
                            cc[:, cs][:, :, :, None].broadcast_to(
                                [128, CH, N_OUT, D_OUT]
                            ),
                        )
                        stmp = spool.tile([128, N_OUT, D_OUT], dt.float32)
                        nc.vector.tensor_reduce(
                            stmp[:],
                            z[:].rearrange("p c o e -> p o e c"),
                            axis=Ax.X,
                            op=Alu.add,
                        )
                        nc.vector.tensor_add(
                            s_part[:],
                            s_part[:],
                            stmp[:].rearrange("p o e -> p (o e)"),
                        )
                    ps_sf = psml.tile([128, 512], dt.float32, tag="ps")
                    nc.tensor.matmul(
                        ps_sf[0:B, 0:OE], e_fold[:], s_part[:], start=True, stop=True
                    )
                    s_psum = ps_sf
                    s_scale = 1.0

                # ---- squash ----
                nc.vector.tensor_scalar_mul(s_sb[:], s_psum[0:B, 0:OE], s_scale)
                nc.vector.tensor_mul(ssq[:], s_sb[:], s_sb[:])
                nc.vector.tensor_reduce(
                    qn[:],
                    ssq[:].rearrange("b (o e) -> b o e", o=N_OUT),
                    axis=Ax.X,
                    op=Alu.add,
                )
                nc.scalar.sqrt(sqn[:], qn[:])
                nc.vector.tensor_scalar_add(den[:], qn[:], 1.0)
                nc.vector.reciprocal(rden[:], den[:])
                nc.vector.tensor_mul(fac[:], sqn[:], rden[:])
                nc.vector.tensor_mul(
                    v_sb[:].rearrange("b (o e) -> b o e", o=N_OUT),
                    s_sb[:].rearrange("b (o e) -> b o e", o=N_OUT),
                    fac[:, :, None].broadcast_to([B, N_OUT, D_OUT]),
                )

                if it == R_ITERS - 1:
                    nc.sync.dma_start(out=p_out[:], in_=v_sb[:])
                else:
                    # replicate v across the 4 partition quadrants (exact fp32)
                    ps_v = psml.tile([128, 512], dt.float32, tag="ps")
                    nc.tensor.matmul(
                        ps_v[:, 0:OE], e_rep[:], v_sb[:], start=True, stop=True
                    )
                    nc.vector.tensor_copy(vrep[:], ps_v[:, 0:OE])
                    # agreement update: bb += sum_e u_ji * v
                    for ch in range(NCH):
                        ts = slice(TCH * ch, TCH * ch + TCH)
                        cs = slice(CH * ch, CH * ch + CH)
                        z2 = zpool.tile([128, CH, N_OUT, D_OUT], dt.bfloat16)
                        nc.vector.tensor_mul(
                            z2[:],
                            Lb[:, ts].rearrange(
                                "p t i (nl o e) -> p (t i nl) o e", nl=NL, o=N_OUT
                            ),
                            vrep[:].rearrange("p (o e) -> p o e", o=N_OUT)[
                                :, None, :, :
                            ].broadcast_to([128, CH, N_OUT, D_OUT]),
                        )
                        bupc = spool.tile([128, CH, N_OUT], dt.float32)
                        nc.vector.tensor_reduce(
                            bupc[:], z2[:], axis=Ax.X, op=Alu.add
                        )
                        nc.vector.tensor_add(bb[:, cs], bb[:, cs], bupc[:])

    nc.compile()
    return nc


def _get_built():
    global _BUILT
    if _BUILT is None:
        _BUILT = _build_bass()
    return _BUILT


def kernel(u_i, weight, bias, r):
    global LAST_EXEC_NS
    assert int(r) == R_ITERS
    u_i = np.asarray(u_i, np.float32)
    weight = np.asarray(weight, np.float32)
    bias = np.asarray(bias, np.float32)

    from concourse.bass_utils import run_bass_kernel_spmd

    w_bd, w_flat, e_fold, e_rep = _host_pack(u_i, weight, bias)
    in_maps = []
    for c in range(N_CORES):
        u_core = u_i[c * B: (c + 1) * B]
        u_pack, u_flat = _host_pack_core(u_core)
        in_maps.append(
            {
                "u_pack": u_pack,
                "u_flat": u_flat,
                "w_bd": w_bd,
                "w_flat": w_flat,
                "e_fold": e_fold,
                "e_rep": e_rep,
            }
        )

    nc = _get_built()
    trace = bool(int(os.environ.get("KERNEL_TRACE", "0")))
    if trace:
        _ensure_ntff_hook()
    res = run_bass_kernel_spmd(nc, in_maps, list(range(N_CORES)), trace=trace)
    LAST_EXEC_NS = getattr(res, "exec_time_ns", None)

    out = np.concatenate(
        [res.results[c]["v_out"].reshape(B, N_OUT, D_OUT) for c in range(N_CORES)],
        axis=0,
    ).astype(np.float32)
    return out


if __name__ == "__main__":
    import reference

    inputs = reference.setup_inputs()
    expected = np.asarray(reference.reference(**inputs))
    actual = kernel(**{k: np.asarray(v) for k, v in inputs.items()})
    err = np.linalg.norm(actual - expected) / np.linalg.norm(expected)
    print("rel err:", err)


# revision 15
# speedup vs baseline: 1.3616x; 1.3616x over previous
"""Capsule dynamic-routing kernel for 8 TRN2 NeuronCores.

Problem: u_ji = einsum('bnd,node->bnoe', u_i, W[0]) + bias ; r=3 rounds of
dynamic routing (softmax over n_out, weighted sum over n_in, squash, agreement
update) ; returns v (batch, n_out, d_out) float32.

Sharding: data-parallel over batch (32 per core), weights replicated.

Device design notes (baseline):
  * n is decomposed as n = 48*t + 12*j + 3*i + nl  (t<24, j<4, i<4, nl<3).
  * The projection runs on TensorE in 32x32 tiling mode. Stationary operand is
    u_i packed per (t,j,i)-group of 3 n's with an augmented contraction row
    d=8 equal to 1.0 so the bias is folded into the matmul. The moving operand
    is a host-materialized block-diagonal W slice. PSUM partitions are (j, b);
    evac writes u_ji to SBUF as bf16 "Lb" [128=(j,b), t, i, nl*160].
  * Iteration 0 uses uniform coupling coefficients, so s_0 is computed exactly
    with one flat K=10368 matmul chain over (n, d9) without touching u_ji.
  * Routing (softmax over o, weighted sums over n, agreement over e) runs on
    DVE/ACT in the (j,b)-partition layout; the 4-way partition fold of
    s-partials and the 4-way replication of v use tiny exact fp32 matmuls with
    0/1 matrices.
"""

import os
import sys
import numpy as np

for _p in ("/opt/trn_rl_repo", "/root/.axon_site/_ro/trn_rl_repo"):
    if os.path.isdir(_p) and _p not in sys.path:
        sys.path.append(_p)

import ml_dtypes

BF16 = ml_dtypes.bfloat16

BATCH, N_IN, D_IN, N_OUT, D_OUT = 256, 1152, 8, 10, 16
R_ITERS = 3
N_CORES = 8
B = BATCH // N_CORES  # 32
D9 = D_IN + 1  # augmented contraction row carries the bias
NT, NJ, NI, NL = 24, 4, 4, 3  # n = 48t + 12j + 3i + nl
OE = N_OUT * D_OUT  # 160
KC = (N_IN * D9) // 128  # 81 flat contraction chunks for s_0
TCH = 2  # t-chunk size for the routing sweeps
NCH = NT // TCH

LAST_EXEC_NS = None

_BUILT = None


def _ensure_ntff_hook():
    """Dev-only: register the NTFF profile hook that bass_utils expects when
    trace=True under axon, if the agent image's antenv lacks it."""
    try:
        from antenv.axon_hooks import get_axon_ntff_profile_hook  # noqa: F401

        return
    except ImportError:
        pass
    import contextlib
    import ctypes
    import types

    so_path = "/opt/axon/libaxon_pjrt.so"
    if not os.path.exists(so_path):
        return
    try:
        lib = ctypes.CDLL(so_path)
        if not hasattr(lib, "axon_start_nrt_profile"):
            return
    except OSError:
        return
    lib.axon_start_nrt_profile.argtypes = [
        ctypes.POINTER(ctypes.c_int64),
        ctypes.c_size_t,
    ]
    lib.axon_start_nrt_profile.restype = ctypes.c_int64
    lib.axon_stop_nrt_profile.argtypes = [ctypes.c_char_p]
    lib.axon_stop_nrt_profile.restype = ctypes.c_int64

    @contextlib.contextmanager
    def _hook(output_dir, device_ids):
        import jax

        jax.devices()
        if device_ids:
            ids = (ctypes.c_int64 * len(device_ids))(*device_ids)
            rc = lib.axon_start_nrt_profile(ids, len(device_ids))
        else:
            rc = lib.axon_start_nrt_profile(None, 0)
        if rc != 0:
            raise RuntimeError(f"axon_start_nrt_profile rc={rc}")
        try:
            yield
        finally:
            n = lib.axon_stop_nrt_profile(str(output_dir).encode())
            print(f"ntff profile: {n} file(s) written to {output_dir}", file=sys.stderr)

    mod = types.ModuleType("antenv.axon_hooks")
    mod._HOOK = _hook
    mod.set_axon_ntff_profile_hook = lambda h: setattr(mod, "_HOOK", h)
    mod.get_axon_ntff_profile_hook = lambda: mod._HOOK
    sys.modules["antenv.axon_hooks"] = mod
    try:
        import antenv

        antenv.axon_hooks = mod
    except ImportError:
        pass


def _host_pack(u_i, weight, bias):
    """Builds the replicated operands (everything except per-core u_i packs)."""
    W = np.asarray(weight)[0]  # (n, o, d, e)
    bias2 = np.asarray(bias)[:, :, 0]  # (n, o)

    # w_bd[32i + 9nl + d, t, j, nl2*160 + o*16 + e]
    Wr = W.reshape(NT, NJ, NI, NL, N_OUT, D_IN, D_OUT)
    br = bias2.reshape(NT, NJ, NI, NL, N_OUT)
    w_bd = np.zeros((NI, 32, NT, NJ, NL, OE), np.float32)
    for nl in range(NL):
        # rows 9*nl + d ; only the nl2 == nl column block is non-zero
        blk = Wr[:, :, :, nl].transpose(2, 4, 0, 1, 3, 5)  # (i, d, t, j, o, e)
        w_bd[:, 9 * nl: 9 * nl + 8, :, :, nl, :] = blk.reshape(NI, 8, NT, NJ, OE)
        bb = br[:, :, :, nl].transpose(2, 0, 1, 3)  # (i, t, j, o)
        w_bd[:, 9 * nl + 8, :, :, nl, :] = np.broadcast_to(
            bb[..., None], (NI, NT, NJ, N_OUT, D_OUT)
        ).reshape(NI, NT, NJ, OE)
    w_bd = w_bd.reshape(128, NT, NJ * NL * OE).astype(BF16)

    # w_flat[k % 128, k // 128, o*16 + e] with k = 9n + d
    wf = np.concatenate(
        [
            W.transpose(0, 2, 1, 3).reshape(N_IN, D_IN, OE),
            np.broadcast_to(bias2[:, None, :, None], (N_IN, 1, N_OUT, D_OUT)).reshape(
                N_IN, 1, OE
            ),
        ],
        axis=1,
    ).reshape(N_IN * D9, OE)
    w_flat = wf.reshape(KC, 128, OE).transpose(1, 0, 2).astype(BF16)

    e_fold = np.zeros((128, B), np.float32)
    for j in range(NJ):
        e_fold[32 * j + np.arange(B), np.arange(B)] = 1.0
    e_rep = np.zeros((B, 128), np.float32)
    for r in range(NJ):
        e_rep[np.arange(B), 32 * r + np.arange(B)] = 1.0
    i128 = np.eye(128, dtype=np.float32)

    return w_bd, w_flat, e_fold.astype(BF16), e_rep, i128.astype(BF16)


def _host_pack_core(u_core):
    """Per-core u_i packs. u_core: (B, n, d) float32."""
    uir = u_core.reshape(B, NT, NJ, NI, NL, D_IN)
    u_pack = np.zeros((NI, 32, NT, NJ, B), np.float32)
    blk = uir.transpose(3, 4, 5, 1, 2, 0)  # (i, nl, d, t, j, b)
    for nl in range(NL):
        u_pack[:, 9 * nl: 9 * nl + 8] = blk[:, nl]
        u_pack[:, 9 * nl + 8] = 1.0
    u_pack = u_pack.reshape(128, NT, NJ, B).astype(BF16)

    uf = np.concatenate(
        [u_core.transpose(1, 2, 0), np.ones((N_IN, 1, B), np.float32)], axis=1
    ).reshape(N_IN * D9, B)
    u_flat = uf.reshape(KC, 128, B).transpose(1, 0, 2).astype(BF16)
    return u_pack, u_flat


def _build_bass():
    import concourse.mybir as mybir
    from concourse import bacc, tile

    dt = mybir.dt
    Alu = mybir.AluOpType
    Act = mybir.ActivationFunctionType
    Ax = mybir.AxisListType

    nc = bacc.Bacc(
        "TRN2", target_bir_lowering=False, debug=False, num_devices=N_CORES
    )

    p_upack = nc.declare_dram_parameter("u_pack", [128, NT, NJ, B], dt.bfloat16, isOutput=False)
    p_uflat = nc.declare_dram_parameter("u_flat", [128, KC, B], dt.bfloat16, isOutput=False)
    p_wbd = nc.declare_dram_parameter("w_bd", [128, NT, NJ * NL * OE], dt.bfloat16, isOutput=False)
    p_wflat = nc.declare_dram_parameter("w_flat", [128, KC, OE], dt.bfloat16, isOutput=False)
    p_efold = nc.declare_dram_parameter("e_fold", [128, B], dt.bfloat16, isOutput=False)
    p_i128 = nc.declare_dram_parameter("i128", [128, 128], dt.bfloat16, isOutput=False)
    p_erep = nc.declare_dram_parameter("e_rep", [B, 128], dt.float32, isOutput=False)
    p_out = nc.declare_dram_parameter("v_out", [B, OE], dt.float32, isOutput=True)

    with tile.TileContext(nc) as tc:
        with (
            tc.tile_pool(name="const", bufs=1) as cpool,
            tc.tile_pool(name="wbd", bufs=3) as wpool,
            tc.tile_pool(name="z", bufs=2) as zpool,
            tc.tile_pool(name="small", bufs=2) as spool,
            tc.tile_pool(name="pproj", bufs=2, space="PSUM") as pp,
            tc.tile_pool(name="psmall", bufs=2, space="PSUM") as psml,
        ):
            # ---- resident inputs ----
            u_pack = cpool.tile([128, NT, NJ, B], dt.bfloat16)
            u_flat = cpool.tile([128, KC, B], dt.bfloat16)
            w_flat = cpool.tile([128, KC, OE], dt.bfloat16)
            e_fold = cpool.tile([128, B], dt.bfloat16)
            i128 = cpool.tile([128, 128], dt.bfloat16)
            e_rep = cpool.tile([B, 128], dt.float32)
            nc.sync.dma_start(out=u_pack[:], in_=p_upack[:])
            nc.sync.dma_start(out=u_flat[:], in_=p_uflat[:])
            nc.sync.dma_start(out=w_flat[:], in_=p_wflat[:])
            nc.sync.dma_start(out=e_fold[:], in_=p_efold[:])
            nc.sync.dma_start(out=i128[:], in_=p_i128[:])
            nc.sync.dma_start(out=e_rep[:], in_=p_erep[:])

            # ---- state ----
            Lb = cpool.tile([128, NT, NI, NL * OE], dt.bfloat16)
            NPP = NT * NI * NL  # 288 n-values per partition
            CH = TCH * NI * NL  # 24 n-values per routing chunk
            bb = cpool.tile([128, NPP, N_OUT], dt.float32)
            expb = cpool.tile([128, NPP, N_OUT], dt.bfloat16)
            cc = cpool.tile([128, NPP, N_OUT], dt.bfloat16)
            zsum = cpool.tile([128, NPP], dt.float32)
            zrec = cpool.tile([128, NPP], dt.float32)
            zrecb = cpool.tile([128, NPP], dt.bfloat16)
            s_sb = cpool.tile([B, OE], dt.float32)
            ssq = cpool.tile([B, OE], dt.float32)
            qn = cpool.tile([B, N_OUT], dt.float32)
            sqn = cpool.tile([B, N_OUT], dt.float32)
            den = cpool.tile([B, N_OUT], dt.float32)
            rden = cpool.tile([B, N_OUT], dt.float32)
            fac = cpool.tile([B, N_OUT], dt.float32)
            v_sb = cpool.tile([B, OE], dt.float32)
            vrep = cpool.tile([128, OE], dt.bfloat16)

            nc.vector.memset(bb[:], 0.0)

            # ---- s_0 = 0.1 * sum_n u_ji  (flat K = 10368 contraction) ----
            ps_s0 = psml.tile([128, 512], dt.float32, tag="aux")
            for k in range(KC):
                nc.tensor.matmul(
                    ps_s0[0:B, 0:OE],
                    u_flat[:, k, :],
                    w_flat[:, k, :],
                    start=(k == 0),
                    stop=(k == KC - 1),
                )

            # ---- projection: u_ji -> Lb ----
            for t in range(NT):
                wbd_t = wpool.tile([128, NJ * NL * OE], dt.bfloat16)
                nc.sync.dma_start(out=wbd_t[:], in_=p_wbd[:, t, :])
                wv = wbd_t[:].rearrange("p (j f) -> p j f", j=NJ)
                for ih in range(2):
                    ps = pp.tile([128, 2, 512], dt.float32)
                    for j in range(NJ):
                        for il in range(2):
                            i = 2 * ih + il
                            nc.tensor.matmul(
                                ps[32 * j: 32 * j + 32, il, 0: NL * OE],
                                u_pack[32 * i: 32 * i + 32, t, j, :],
                                wv[32 * i: 32 * i + 32, j, :],
                                start=True,
                                stop=True,
                                tile_position=(32 * i, 32 * j),
                            )
                    eng = nc.vector if (2 * t + ih) % 2 == 0 else nc.scalar
                    if eng is nc.vector:
                        nc.vector.tensor_copy(
                            Lb[:, t, 2 * ih: 2 * ih + 2, :], ps[:, :, 0: NL * OE]
                        )
                    else:
                        nc.scalar.copy(
                            Lb[:, t, 2 * ih: 2 * ih + 2, :], ps[:, :, 0: NL * OE]
                        )

            # ---- routing iterations ----
            for it in range(R_ITERS):
                if it == 0:
                    s_psum = ps_s0
                    s_scale = 0.1
                else:
                    # softmax over o (free dim), then s = sum_n c * u_ji
                    nc.scalar.activation(expb[:], bb[:], Act.Exp)
                    nc.vector.tensor_reduce(zsum[:], expb[:], axis=Ax.X, op=Alu.add)
                    nc.vector.reciprocal(zrec[:], zsum[:])
                    nc.vector.tensor_copy(zrecb[:], zrec[:])
                    nc.vector.tensor_mul(
                        cc[:],
                        expb[:],
                        zrecb[:, :, None].broadcast_to([128, NPP, N_OUT]),
                    )
                    ps_sf = psml.tile([128, 512], dt.float32, tag="acc")
                    for ch in range(NCH):
                        ts = slice(TCH * ch, TCH * ch + TCH)
                        cs = slice(CH * ch, CH * ch + CH)
                        z = zpool.tile([128, CH, N_OUT, D_OUT], dt.bfloat16)
                        nc.vector.tensor_mul(
                            z[:],
                            Lb[:, ts].rearrange(
                                "p t i (nl o e) -> p (t i nl) o e", nl=NL, o=N_OUT
                            ),
                            cc[:, cs][:, :, :, None].broadcast_to(
                                [128, CH, N_OUT, D_OUT]
                            ),
                        )
                        # s[b] += sum_j z[(j,b), n_loc, :, :]  via fold matmuls
                        for nl2 in range(CH):
                            nc.tensor.matmul(
                                ps_sf[0:B, 0:OE],
                                e_fold[:],
                                z[:, nl2, :, :],
                                start=(ch == 0 and nl2 == 0),
                                stop=(ch == NCH - 1 and nl2 == CH - 1),
                            )
                    s_psum = ps_sf
                    s_scale = 1.0

                # ---- squash ----
                nc.vector.tensor_scalar_mul(s_sb[:], s_psum[0:B, 0:OE], s_scale)
                nc.vector.tensor_mul(ssq[:], s_sb[:], s_sb[:])
                nc.vector.tensor_reduce(
                    qn[:],
                    ssq[:].rearrange("b (o e) -> b o e", o=N_OUT),
                    axis=Ax.X,
                    op=Alu.add,
                )
                nc.scalar.sqrt(sqn[:], qn[:])
                nc.vector.tensor_scalar_add(den[:], qn[:], 1.0)
                nc.vector.reciprocal(rden[:], den[:])
                nc.vector.tensor_mul(fac[:], sqn[:], rden[:])
                nc.vector.tensor_mul(
                    v_sb[:].rearrange("b (o e) -> b o e", o=N_OUT),
                    s_sb[:].rearrange("b (o e) -> b o e", o=N_OUT),
                    fac[:, :, None].broadcast_to([B, N_OUT, D_OUT]),
                )

                if it == R_ITERS - 1:
                    nc.sync.dma_start(out=p_out[:], in_=v_sb[:])
                else:
                    # replicate v across the 4 partition quadrants (exact fp32)
                    ps_v = psml.tile([128, 512], dt.float32, tag="aux")
                    nc.tensor.matmul(
                        ps_v[:, 0:OE], e_rep[:], v_sb[:], start=True, stop=True
                    )
                    nc.vector.tensor_copy(vrep[:], ps_v[:, 0:OE])
                    # agreement update: bb += sum_e u_ji * v
                    for ch in range(NCH):
                        ts = slice(TCH * ch, TCH * ch + TCH)
                        cs = slice(CH * ch, CH * ch + CH)
                        z2 = zpool.tile([128, CH, N_OUT, D_OUT], dt.bfloat16)
                        nc.vector.tensor_mul(
                            z2[:],
                            Lb[:, ts].rearrange(
                                "p t i (nl o e) -> p (t i nl) o e", nl=NL, o=N_OUT
                            ),
                            vrep[:].rearrange("p (o e) -> p o e", o=N_OUT)[
                                :, None, :, :
                            ].broadcast_to([128, CH, N_OUT, D_OUT]),
                        )
                        # bup[(j,b), n, o] = sum_e z2  via identity matmuls
                        pb = psml.tile([128, CH * N_OUT], dt.float32, tag="acc")
                        for e in range(D_OUT):
                            nc.tensor.matmul(
                                pb[:, 0: CH * N_OUT],
                                i128[:],
                                z2[:, :, :, e],
                                start=(e == 0),
                                stop=(e == D_OUT - 1),
                            )
                        nc.vector.tensor_add(
                            bb[:, cs],
                            bb[:, cs],
                            pb[:].rearrange("p (c o) -> p c o", o=N_OUT),
                        )

    nc.compile()
    return nc


def _get_built():
    global _BUILT
    if _BUILT is None:
        _BUILT = _build_bass()
    return _BUILT


def kernel(u_i, weight, bias, r):
    global LAST_EXEC_NS
    assert int(r) == R_ITERS
    u_i = np.asarray(u_i, np.float32)
    weight = np.asarray(weight, np.float32)
    bias = np.asarray(bias, np.float32)

    from concourse.bass_utils import run_bass_kernel_spmd

    w_bd, w_flat, e_fold, e_rep, i128 = _host_pack(u_i, weight, bias)
    in_maps = []
    for c in range(N_CORES):
        u_core = u_i[c * B: (c + 1) * B]
        u_pack, u_flat = _host_pack_core(u_core)
        in_maps.append(
            {
                "u_pack": u_pack,
                "u_flat": u_flat,
                "w_bd": w_bd,
                "w_flat": w_flat,
                "e_fold": e_fold,
                "e_rep": e_rep,
                "i128": i128,
            }
        )

    nc = _get_built()
    trace = bool(int(os.environ.get("KERNEL_TRACE", "0")))
    if trace:
        _ensure_ntff_hook()
    res = run_bass_kernel_spmd(nc, in_maps, list(range(N_CORES)), trace=trace)
    LAST_EXEC_NS = getattr(res, "exec_time_ns", None)

    out = np.concatenate(
        [res.results[c]["v_out"].reshape(B, N_OUT, D_OUT) for c in range(N_CORES)],
        axis=0,
    ).astype(np.float32)
    return out


if __name__ == "__main__":
    import reference

    inputs = reference.setup_inputs()
    expected = np.asarray(reference.reference(**inputs))
    actual = kernel(**{k: np.asarray(v) for k, v in inputs.items()})
    err = np.linalg.norm(actual - expected) / np.linalg.norm(expected)
    print("rel err:", err)
